# revision 28
# baseline (speedup 1.0000x reference)
"""BinLinear Trainium2 kernel: out = x @ sign(W)^T + sign(bias).

Full shapes: x [8192, 4096] f32, W [4096, 4096] f32, bias [4096] f32,
out [8192, 4096] f32. 8 NeuronCores, data-parallel on the token dim M:
core i gets x[1024*i:1024*(i+1)], full W/bias; host concatenates outputs.

Design (measured 0.996 ms vs the 1.54 ms hi/lo baseline; rel err 1.7e-3,
tolerance 2e-2):
  - ONE bf16 matmul pass: x rounded to bf16 (rel ~1e-3), sign(W) exact in
    bf16, fp32 PSUM accumulation. Per (kt, mi): moving = sign(W)^T tile
    [128,512], stationary = x^T tile [128,128]; per-strip rank-1 bias
    matmul (DVE-memset ones row x bit-tricked sign(bias) row) clears and
    seeds each PSUM bank.
  - Software-pipelined emission (skew 1, consume-before-load) so each
    strip's PSUM evictions land in DVE program order right after that
    strip's last transpose; per-bank staggered eviction keeps strip
    boundaries to ~1-2us of PE idle.
  - W pipeline per tile, all off the critical ACT path: 4 swizzled DMAs
    (3-dim APs; queue alternates Scalar/GpSimd per tile, never split within
    a tile - a recycled slot's WAW only collapses for same-queue writers),
    4 one-elem DVE lane touches, an in-place DVE bitwise sign
    ((w & 0x80000000) | 0x3F800000 == +-1.0f, reading every staged byte so
    the recycling DMA has a single collapsible dep), and a strided-u16
    32x32 stream transpose of the +-1.0f high half-words -> bf16 W^T.
  - x^T is built the same way during strip 0 (swizzle -> DVE round-copy ->
    transpose) into a resident [128, 32, 1024] bf16 tensor.
  - walrus allows ONE sync wait per instruction, and Tile emits waits for
    every tile-mediated dep it cannot prove covered by the engine's
    already-waited clock. All cross-engine coupling therefore runs through
    write-once observer scratches with forced sync edges (_observe):
    GpSimd/ACT observe the exact transpose whose staging slot their next
    DMA recycles; DVE observes the exact matmul whose wtt slot the next
    transpose overwrites; per-bank ACT eclaims observe each eviction copy
    so each out-DMA elides its data wait and keeps only its lane wait.
"""

import numpy as np

import concourse.bass as bass
import concourse.mybir as mybir
import concourse.tile as tile
from concourse.vector_clock import ScopedClock, VectorClock
from concourse.tile import add_dep_helper
from concourse.bass_utils import run_bass_kernel_spmd


class SplitDrainTileContext(tile.TileContext):
    """TileContext whose kernel-tail drain is split into several drain
    instructions. The stock tail emits ONE drain waiting on every active proc
    (engines + all DMA lanes, ~15 waits) which overflows the CTRL
    instruction's sync-wait slots in walrus codegen. Emitting the same waits
    across several drains (<= 4 waits each) is semantically identical: each
    drain's waits are satisfied in turn and the final state is 'everything
    quiesced'."""

    MAX_DRAIN_WAITS = 1

    def _drain_and_barrier(self, tick_clock, wait_clock):
        gc = tick_clock.global_clock
        n = len(gc)
        for lo in range(0, n, self.MAX_DRAIN_WAITS):
            vc = VectorClock()
            for p in range(lo, min(lo + self.MAX_DRAIN_WAITS, n)):
                if gc[p]:
                    vc.require_at_least(p, gc[p])
            drain_inst = self.nc.sync.drain()
            wait_clock.add_sem_waits(
                drain_inst.ins, ScopedClock({None: vc})
            )
        self.nc.all_engine_barrier()
        assert self.sems is not None
        popped = self.nc._tile_sem_poison_stack.pop()
        assert popped is self._sem_poison
        self.nc.clear_and_free_semaphores(list(self.sems.allocated().values()))
        self.nc.all_engine_barrier()


P = 128
NFREE = 512  # moving free dim per matmul (one PSUM bank of fp32)

M_FULL, K_FULL, N_FULL = 8192, 4096, 4096
N_CORES = 8
M_SHARD = M_FULL // N_CORES

# sign bit-trick masks
SIGN_AND = 0x8000
SIGN_OR = 0x3F80  # 1.0 in bf16
SIGN_AND32 = 0x80000000
SIGN_OR32 = 0x3F800000  # 1.0 in f32


def _swizzled_load(engine, sbuf_tile, dram_ap):
    """Load dram_ap ([R, 128] slice) into sbuf_tile [128, R] block-swizzled so
    that a DVE 32x32 stream transpose of sbuf_tile yields dram_ap.T.

    Pre-DVE we need:  sbuf[32g+a, 32b+c] = dram[32b+a, 32g+c]
    so post-DVE:      out[32g+a, 32b+c] = dram[32b+c, 32g+a] = dram.T[p, f].

    DMA access patterns are limited to 3 dims, so issue one DMA per
    partition-group g (source dims [a, b, c], 128-byte contiguous runs).

    ALL four DMAs must come from the SAME queue: a recycled slot's new DMA
    carries WAW waits vs the old tile's writers, and only same-queue lane
    ticks are covered by the issuing queue's own lane-wait chain (cross-queue
    lane sems would each cost a sync-wait slot the DMA doesn't have).
    """
    first = None
    for g in range(4):
        di = engine.dma_start(
            sbuf_tile[32 * g : 32 * (g + 1), :],
            dram_ap[:, 32 * g : 32 * (g + 1)].rearrange("(b a) c -> a b c", a=32),
        )
        if first is None:
            first = di
    return first


def _touch4(nc, sbuf_tile):
    """In-place 1-element DVE copies, one per partition group. Each waits on
    one of the 4 swizzle DMAs, advancing the DVE's observed semaphore ticks so
    the full-width consumer that follows needs no waits of its own (the HW
    allows only a few sync-wait commands per instruction)."""
    for g in range(4):
        s = sbuf_tile[32 * g : 32 * (g + 1), 0:1]
        nc.vector.tensor_copy(out=s, in_=s)


def _observe(eng_memset_or_act, scr, anchor_inst, reason):
    """Advance a queue's observed clock past `anchor_inst` without touching
    any real data tile: a write-once 1-elem scratch write plus a forced
    sync edge. The write-once target means no WAW; the single forced wait is
    the instruction's only one, and later same-queue instructions elide any
    dep at or before the anchor's tick. Returns the observer instruction."""
    inst = eng_memset_or_act(scr)
    add_dep_helper(inst.ins, anchor_inst.ins, sync=True, reason=reason)
    return inst


def bin_linear_tile_kernel(tc, x_ap, w_ap, b_ap, o_ap):
    nc = tc.nc
    f32 = mybir.dt.float32
    bf16 = mybir.dt.bfloat16
    u16 = mybir.dt.uint16
    u32 = mybir.dt.uint32
    AND = mybir.AluOpType.bitwise_and
    OR = mybir.AluOpType.bitwise_or
    COPY = mybir.ActivationFunctionType.Copy

    MS, K = x_ap.shape  # m per core, contraction
    N = w_ap.shape[0]
    KT = K // P  # k tiles
    MT = MS // P  # m tiles (psum banks used per n-strip)
    NS = N // NFREE  # n strips
    NT = NS * KT  # total W tiles
    SKEW = 1  # load-ahead: W tile t is loaded SKEW iterations before its MMs
    # (small on purpose: the DMA queues already run ahead via the staging
    # bufs; a bigger skew just pushes each strip's eviction copies later in
    # DVE program order, stretching the strip-boundary PE gap and
    # re-throttling HAM.)
    WSZ_BUFS = 10  # even: a recycled slot's old DMA writers are same-queue
    WTT_BUFS = 16
    XS_BUFS = 3
    assert MT <= 8, "psum accumulators exceed the 8 PSUM banks"

    with (
        tc.tile_pool(name="xt", bufs=1) as xt_pool,
        tc.tile_pool(name="xstg", bufs=2) as xstg_pool,
        tc.tile_pool(name="wstg", bufs=2) as wstg_pool,
        tc.tile_pool(name="outp", bufs=1) as out_pool,
        tc.tile_pool(name="bias", bufs=1) as bias_pool,
        tc.tile_pool(name="obs", bufs=1) as obs_pool,
        tc.tile_pool(name="psum", bufs=8, space="PSUM") as psum_pool,
    ):
        # Write-once observer scratches (see _observe). Unique cells: a
        # rotating scratch's WAW would cost a second wait on engines whose
        # own-sem clock never advances (Pool/ACT).
        nobs = [0]

        def gp_observe(anchor, reason):
            scr = obs_pool.tile([1, 1], f32, name=f"gsc{nobs[0]}")
            nobs[0] += 1
            return _observe(
                lambda s: nc.gpsimd.memset(s[:], 0.0), scr, anchor, reason
            )

        def dve_observe(anchor, reason):
            scr = obs_pool.tile([1, 1], f32, name=f"dsc{nobs[0]}")
            nobs[0] += 1
            return _observe(
                lambda s: nc.vector.memset(s[:], 0.0), scr, anchor, reason
            )

        # --- bias: sign via the DVE bit trick; rank-1 matmul operands.
        bstg = bias_pool.tile([1, N], f32, name="bstg")
        nc.sync.dma_start(bstg[:], b_ap[None, :])
        s = bstg[0:1, 0:1]
        nc.vector.tensor_copy(out=s, in_=s)
        bias_sgn = bias_pool.tile([1, N], bf16, name="bias_sgn")
        nc.vector.tensor_scalar(
            out=bias_sgn[:].bitcast(u16),
            in0=bstg[:].bitcast(u16)[:, 1::2],
            scalar1=SIGN_AND,
            scalar2=SIGN_OR,
            op0=AND,
            op1=OR,
        )
        ones_row = bias_pool.tile([1, P], bf16, name="ones_row")
        nc.vector.memset(ones_row[:], 1.0)

        def act_observe(anchor, reason):
            # ACT observer: 1-elem activation copy from the never-rewritten
            # ones_row into a write-once scratch; the forced DVE edge merges
            # with the (ancient) ones_row RAW into a single DVE wait.
            scr = obs_pool.tile([1, 1], f32, name=f"asc{nobs[0]}")
            nobs[0] += 1
            inst = nc.scalar.activation(scr[:], ones_row[0:1, 0:1], COPY)
            add_dep_helper(inst.ins, anchor.ins, sync=True, reason=reason)
            return inst

        # x^T resident: [128, KT, MS] bf16
        xt = xt_pool.tile([P, KT, MS], bf16, name="xt")
        # out staging: one [128, MT*NFREE] tile per strip, written by the MT
        # eviction copies, drained by ONE 3D out-DMA (dst dims [mi, p, n]).
        ot_big = out_pool.tile([P, MT, NFREE], f32, name="ot_big")

        psums = [
            psum_pool.tile([P, NFREE], f32, name=f"psum_{mi}", tag="acc")
            for mi in range(MT)
        ]

        tr_hist = []  # wtt transpose instruction per W-tile index
        xcp_hist = []  # xsb-copy instruction per x tile
        mm_last = []  # last matmul instruction per W-tile index
        last_act_obs = None
        last_gp_obs = None
        last_eclaim = None
        wtts = {}  # live wtt tiles by tile index

        def load_tile(t):
            nonlocal last_act_obs, last_gp_obs
            ns, kt = divmod(t, KT)
            nlo = ns * NFREE
            gp_parity = t % 2 == 1  # odd W tiles load via the GpSimd queue
            # Observers anchor on the EXACT instruction whose tick the next
            # DMA's WAR needs: the transpose that read the recycled wsz slot.
            if t >= WSZ_BUFS:
                if gp_parity:
                    last_gp_obs = gp_observe(tr_hist[t - WSZ_BUFS], "gp clock")
                else:
                    last_act_obs = act_observe(tr_hist[t - WSZ_BUFS], "act clock")
            if ns == 0 and kt >= XS_BUFS:
                # x staging WAR: the xsb copy that read xs(kt-XS_BUFS).
                last_act_obs = act_observe(xcp_hist[kt - XS_BUFS], "act x clock")
            if ns == 0:
                # x prologue interleaved with strip 0 (Scalar queue).
                xs = xstg_pool.tile(
                    [P, MS], f32, name=f"xs{kt}", tag="xs", bufs=XS_BUFS
                )
                first = _swizzled_load(nc.scalar, xs, x_ap[:, kt * P : (kt + 1) * P])
                if last_act_obs is not None:
                    add_dep_helper(
                        first.ins, last_act_obs.ins, sync=False, reason="x after obs"
                    )
                _touch4(nc, xs)
                xsb = xstg_pool.tile([P, MS], bf16, name=f"xsb{kt}", tag="xsb", bufs=2)
                xcp = nc.vector.tensor_copy(out=xsb[:], in_=xs[:])  # ->bf16
                xcp_hist.append(xcp)
                nc.vector.transpose(xt[:, kt, :], xsb[:])
            # W tile: swizzle DMAs on one queue (alternating per tile), then
            # touch4 -> in-place bitwise sign -> strided-u16 transpose on DVE.
            wsz = wstg_pool.tile(
                [P, NFREE], f32, name=f"wsz_{t}", tag="wsz", bufs=WSZ_BUFS
            )
            first = _swizzled_load(
                nc.gpsimd if gp_parity else nc.scalar,
                wsz,
                w_ap[nlo : nlo + NFREE, kt * P : (kt + 1) * P],
            )
            pin = last_gp_obs if gp_parity else last_act_obs
            if pin is not None:
                add_dep_helper(first.ins, pin.ins, sync=False, reason="dma after obs")
            _touch4(nc, wsz)
            # in-place sign: (w & 0x80000000) | 0x3F800000 == +-1.0f. Reads
            # AND writes every staged byte, so the recycling DMA's deps
            # collapse into one DVE tick (<= the transpose read below).
            nc.vector.tensor_scalar(
                out=wsz[:].bitcast(u32),
                in0=wsz[:].bitcast(u32),
                scalar1=SIGN_AND32,
                scalar2=SIGN_OR32,
                op0=AND,
                op1=OR,
            )
            wtt = wstg_pool.tile(
                [P, NFREE], bf16, name=f"wtt_{t}", tag="wtt", bufs=WTT_BUFS
            )
            if t >= WTT_BUFS:
                # DVE observes PE past the matmuls that read the recycled wtt
                # slot, so the transpose keeps only its own-queue (sign) wait.
                dob = dve_observe(mm_last[t - WTT_BUFS], "dve sees pe")
            tr = nc.vector.transpose(
                wtt[:].bitcast(u16), wsz[:].bitcast(u16)[:, 1::2]
            )
            if t >= WTT_BUFS:
                add_dep_helper(
                    tr.ins, dob.ins, sync=False, reason="transpose after pe obs"
                )
            tr_hist.append(tr)
            wtts[t] = wtt

        def consume_tile(t):
            nonlocal last_eclaim
            ns, kt = divmod(t, KT)
            nlo = ns * NFREE
            if kt == 0:
                # bias enters PSUM first: rank-1 matmul, start=True clears
                # the bank; waits only bank mi's eviction copy (DVE).
                for mi in range(MT):
                    nc.tensor.matmul(
                        psums[mi][:],
                        ones_row[:],
                        bias_sgn[:, nlo : nlo + NFREE],
                        start=True,
                        stop=False,
                    )
            wtt = wtts.pop(t)
            last = kt == KT - 1
            for mi in range(MT):
                mm = nc.tensor.matmul(
                    psums[mi][:],
                    xt[:, kt, mi * P : (mi + 1) * P],
                    wtt[:],
                    start=False,
                    stop=last,
                )
            mm_last.append(mm)
            if last:
                # Staggered per-bank eviction into ot_big slices. Each bank's
                # out-DMA follows its OWN ACT observe (anchored on that
                # bank's copy), so no cross-copy scheduling assumption is
                # load-bearing: the DMA's data wait elides against a tick
                # that provably covers exactly the slice it reads.
                for mi in range(MT):
                    s = psums[mi][0:1, 0:1]
                    nc.vector.tensor_copy(out=s, in_=s)
                    cp = nc.vector.tensor_copy(
                        out=ot_big[:, mi, :], in_=psums[mi][:]
                    )
                    ecl = act_observe(cp, "eclaim")
                    di = nc.scalar.dma_start(
                        o_ap[mi * P : (mi + 1) * P, nlo : nlo + NFREE],
                        ot_big[:, mi, :],
                    )
                    add_dep_helper(
                        di.ins, ecl.ins, sync=False, reason="out after eclaim"
                    )

        for t in range(NT + SKEW):
            # consume first so a strip's eviction copies land in DVE program
            # order right after that strip's last transpose, not behind the
            # next strip's staging work.
            if t >= SKEW:
                consume_tile(t - SKEW)
            if t < NT:
                load_tile(t)


def build_module(m_shard=M_SHARD, k=K_FULL, n=N_FULL):
    nc = bass.Bass("TRN2", target_bir_lowering=False, debug=False)
    f32 = mybir.dt.float32
    x_d = nc.dram_tensor("x", [m_shard, k], f32, kind="ExternalInput")
    w_d = nc.dram_tensor("weight", [n, k], f32, kind="ExternalInput")
    b_d = nc.dram_tensor("bias", [n], f32, kind="ExternalInput")
    o_d = nc.dram_tensor("out", [m_shard, n], f32, kind="ExternalOutput")
    with SplitDrainTileContext(nc) as tc:
        bin_linear_tile_kernel(tc, x_d.ap(), w_d.ap(), b_d.ap(), o_d.ap())
    return nc


_NC_CACHE = {}


def _get_module():
    if "nc" not in _NC_CACHE:
        _NC_CACHE["nc"] = build_module()
    return _NC_CACHE["nc"]


def make_in_maps(x, weight, bias):
    x = np.ascontiguousarray(np.asarray(x, dtype=np.float32))
    weight = np.ascontiguousarray(np.asarray(weight, dtype=np.float32))
    bias = np.ascontiguousarray(np.asarray(bias, dtype=np.float32))
    return [
        {
            "x": x[i * M_SHARD : (i + 1) * M_SHARD],
            "weight": weight,
            "bias": bias,
        }
        for i in range(N_CORES)
    ]


def gather(results):
    return np.concatenate([results[i]["out"] for i in range(N_CORES)], axis=0)


def run(x, weight, bias, trace=False, **kw):
    """Run on the 8 NeuronCores; returns (out_full, BassKernelResults)."""
    nc = _get_module()
    in_maps = make_in_maps(x, weight, bias)
    res = run_bass_kernel_spmd(nc, in_maps, list(range(N_CORES)), trace=trace, **kw)
    return gather(res.results), res


def kernel(x, weight, bias):
    out, _ = run(x, weight, bias)
    return out


# revision 31
# speedup vs baseline: 1.0086x; 1.0086x over previous
"""BinLinear Trainium2 kernel: out = x @ sign(W)^T + sign(bias).

Full shapes: x [8192, 4096] f32, W [4096, 4096] f32, bias [4096] f32,
out [8192, 4096] f32. 8 NeuronCores, data-parallel on the token dim M:
core i gets x[1024*i:1024*(i+1)], full W/bias; host concatenates outputs.

Design (measured 0.996 ms vs the 1.54 ms hi/lo baseline; rel err 1.7e-3,
tolerance 2e-2):
  - ONE bf16 matmul pass: x rounded to bf16 (rel ~1e-3), sign(W) exact in
    bf16, fp32 PSUM accumulation. Per (kt, mi): moving = sign(W)^T tile
    [128,512], stationary = x^T tile [128,128]; per-strip rank-1 bias
    matmul (DVE-memset ones row x bit-tricked sign(bias) row) clears and
    seeds each PSUM bank.
  - Software-pipelined emission (skew 1, consume-before-load) so each
    strip's PSUM evictions land in DVE program order right after that
    strip's last transpose; per-bank staggered eviction keeps strip
    boundaries to ~1-2us of PE idle.
  - W pipeline per tile, all off the critical ACT path: 4 swizzled DMAs
    (3-dim APs; queue alternates Scalar/GpSimd per tile, never split within
    a tile - a recycled slot's WAW only collapses for same-queue writers),
    4 one-elem DVE lane touches, an in-place DVE bitwise sign
    ((w & 0x80000000) | 0x3F800000 == +-1.0f, reading every staged byte so
    the recycling DMA has a single collapsible dep), and a strided-u16
    32x32 stream transpose of the +-1.0f high half-words -> bf16 W^T.
  - x^T is built the same way during strip 0 (swizzle -> DVE round-copy ->
    transpose) into a resident [128, 32, 1024] bf16 tensor.
  - walrus allows ONE sync wait per instruction, and Tile emits waits for
    every tile-mediated dep it cannot prove covered by the engine's
    already-waited clock. All cross-engine coupling therefore runs through
    write-once observer scratches with forced sync edges (_observe):
    GpSimd/ACT observe the exact transpose whose staging slot their next
    DMA recycles; DVE observes the exact matmul whose wtt slot the next
    transpose overwrites; per-bank ACT eclaims observe each eviction copy
    so each out-DMA elides its data wait and keeps only its lane wait.
"""

import numpy as np

import concourse.bass as bass
import concourse.mybir as mybir
import concourse.tile as tile
from concourse.vector_clock import ScopedClock, VectorClock
from concourse.tile import add_dep_helper
from concourse.bass_utils import run_bass_kernel_spmd


class SplitDrainTileContext(tile.TileContext):
    """TileContext whose kernel-tail drain is split into several drain
    instructions. The stock tail emits ONE drain waiting on every active proc
    (engines + all DMA lanes, ~15 waits) which overflows the CTRL
    instruction's sync-wait slots in walrus codegen. Emitting the same waits
    across several drains (<= 4 waits each) is semantically identical: each
    drain's waits are satisfied in turn and the final state is 'everything
    quiesced'."""

    MAX_DRAIN_WAITS = 1

    def _drain_and_barrier(self, tick_clock, wait_clock):
        gc = tick_clock.global_clock
        n = len(gc)
        for lo in range(0, n, self.MAX_DRAIN_WAITS):
            vc = VectorClock()
            for p in range(lo, min(lo + self.MAX_DRAIN_WAITS, n)):
                if gc[p]:
                    vc.require_at_least(p, gc[p])
            drain_inst = self.nc.sync.drain()
            wait_clock.add_sem_waits(
                drain_inst.ins, ScopedClock({None: vc})
            )
        self.nc.all_engine_barrier()
        assert self.sems is not None
        popped = self.nc._tile_sem_poison_stack.pop()
        assert popped is self._sem_poison
        self.nc.clear_and_free_semaphores(list(self.sems.allocated().values()))
        self.nc.all_engine_barrier()


P = 128
NFREE = 512  # moving free dim per matmul (one PSUM bank of fp32)

M_FULL, K_FULL, N_FULL = 8192, 4096, 4096
N_CORES = 8
M_SHARD = M_FULL // N_CORES

# sign bit-trick masks
SIGN_AND = 0x8000
SIGN_OR = 0x3F80  # 1.0 in bf16
SIGN_AND32 = 0x80000000
SIGN_OR32 = 0x3F800000  # 1.0 in f32


def _swizzled_load(engine, sbuf_tile, dram_ap):
    """Load dram_ap ([R, 128] slice) into sbuf_tile [128, R] block-swizzled so
    that a DVE 32x32 stream transpose of sbuf_tile yields dram_ap.T.

    Pre-DVE we need:  sbuf[32g+a, 32b+c] = dram[32b+a, 32g+c]
    so post-DVE:      out[32g+a, 32b+c] = dram[32b+c, 32g+a] = dram.T[p, f].

    DMA access patterns are limited to 3 dims, so issue one DMA per
    partition-group g (source dims [a, b, c], 128-byte contiguous runs).

    ALL four DMAs must come from the SAME queue: a recycled slot's new DMA
    carries WAW waits vs the old tile's writers, and only same-queue lane
    ticks are covered by the issuing queue's own lane-wait chain (cross-queue
    lane sems would each cost a sync-wait slot the DMA doesn't have).
    """
    first = None
    for g in range(4):
        di = engine.dma_start(
            sbuf_tile[32 * g : 32 * (g + 1), :],
            dram_ap[:, 32 * g : 32 * (g + 1)].rearrange("(b a) c -> a b c", a=32),
        )
        if first is None:
            first = di
    return first


def _touch4(nc, sbuf_tile):
    """In-place 1-element DVE copies, one per partition group. Each waits on
    one of the 4 swizzle DMAs, advancing the DVE's observed semaphore ticks so
    the full-width consumer that follows needs no waits of its own (the HW
    allows only a few sync-wait commands per instruction)."""
    for g in range(4):
        s = sbuf_tile[32 * g : 32 * (g + 1), 0:1]
        nc.vector.tensor_copy(out=s, in_=s)


def _observe(eng_memset_or_act, scr, anchor_inst, reason):
    """Advance a queue's observed clock past `anchor_inst` without touching
    any real data tile: a write-once 1-elem scratch write plus a forced
    sync edge. The write-once target means no WAW; the single forced wait is
    the instruction's only one, and later same-queue instructions elide any
    dep at or before the anchor's tick. Returns the observer instruction."""
    inst = eng_memset_or_act(scr)
    add_dep_helper(inst.ins, anchor_inst.ins, sync=True, reason=reason)
    return inst


def bin_linear_tile_kernel(tc, x_ap, w_ap, b_ap, o_ap):
    nc = tc.nc
    f32 = mybir.dt.float32
    bf16 = mybir.dt.bfloat16
    u16 = mybir.dt.uint16
    u32 = mybir.dt.uint32
    AND = mybir.AluOpType.bitwise_and
    OR = mybir.AluOpType.bitwise_or
    COPY = mybir.ActivationFunctionType.Copy

    MS, K = x_ap.shape  # m per core, contraction
    N = w_ap.shape[0]
    KT = K // P  # k tiles
    MT = MS // P  # m tiles (psum banks used per n-strip)
    NS = N // NFREE  # n strips
    NT = NS * KT  # total W tiles
    SKEW = 1  # load-ahead: W tile t is loaded SKEW iterations before its MMs
    # (small on purpose: the DMA queues already run ahead via the staging
    # bufs; a bigger skew just pushes each strip's eviction copies later in
    # DVE program order, stretching the strip-boundary PE gap and
    # re-throttling HAM.)
    WSZ_BUFS = 10  # even: a recycled slot's old DMA writers are same-queue
    WTT_BUFS = 24
    XS_BUFS = 3
    assert MT <= 8, "psum accumulators exceed the 8 PSUM banks"

    with (
        tc.tile_pool(name="xt", bufs=1) as xt_pool,
        tc.tile_pool(name="xstg", bufs=2) as xstg_pool,
        tc.tile_pool(name="wstg", bufs=2) as wstg_pool,
        tc.tile_pool(name="outp", bufs=1) as out_pool,
        tc.tile_pool(name="bias", bufs=1) as bias_pool,
        tc.tile_pool(name="obs", bufs=1) as obs_pool,
        tc.tile_pool(name="psum", bufs=8, space="PSUM") as psum_pool,
    ):
        # Write-once observer scratches (see _observe). Unique cells: a
        # rotating scratch's WAW would cost a second wait on engines whose
        # own-sem clock never advances (Pool/ACT).
        nobs = [0]

        def gp_observe(anchor, reason):
            scr = obs_pool.tile([1, 1], f32, name=f"gsc{nobs[0]}")
            nobs[0] += 1
            return _observe(
                lambda s: nc.gpsimd.memset(s[:], 0.0), scr, anchor, reason
            )

        def dve_observe(anchor, reason):
            scr = obs_pool.tile([1, 1], f32, name=f"dsc{nobs[0]}")
            nobs[0] += 1
            return _observe(
                lambda s: nc.vector.memset(s[:], 0.0), scr, anchor, reason
            )

        # --- bias: sign via the DVE bit trick; rank-1 matmul operands.
        bstg = bias_pool.tile([1, N], f32, name="bstg")
        nc.sync.dma_start(bstg[:], b_ap[None, :])
        s = bstg[0:1, 0:1]
        nc.vector.tensor_copy(out=s, in_=s)
        bias_sgn = bias_pool.tile([1, N], bf16, name="bias_sgn")
        nc.vector.tensor_scalar(
            out=bias_sgn[:].bitcast(u16),
            in0=bstg[:].bitcast(u16)[:, 1::2],
            scalar1=SIGN_AND,
            scalar2=SIGN_OR,
            op0=AND,
            op1=OR,
        )
        ones_row = bias_pool.tile([1, P], bf16, name="ones_row")
        nc.vector.memset(ones_row[:], 1.0)

        def act_observe(anchor, reason):
            # ACT observer: 1-elem activation copy from the never-rewritten
            # ones_row into a write-once scratch; the forced DVE edge merges
            # with the (ancient) ones_row RAW into a single DVE wait.
            scr = obs_pool.tile([1, 1], f32, name=f"asc{nobs[0]}")
            nobs[0] += 1
            inst = nc.scalar.activation(scr[:], ones_row[0:1, 0:1], COPY)
            add_dep_helper(inst.ins, anchor.ins, sync=True, reason=reason)
            return inst

        # x^T resident: [128, KT, MS] bf16
        xt = xt_pool.tile([P, KT, MS], bf16, name="xt")
        # out staging: one [128, MT*NFREE] tile per strip, written by the MT
        # eviction copies, drained by ONE 3D out-DMA (dst dims [mi, p, n]).
        ot_big = out_pool.tile([P, MT, NFREE], f32, name="ot_big")

        psums = [
            psum_pool.tile([P, NFREE], f32, name=f"psum_{mi}", tag="acc")
            for mi in range(MT)
        ]

        tr_hist = []  # wtt transpose instruction per W-tile index
        xcp_hist = []  # xsb-copy instruction per x tile
        mm_last = []  # last matmul instruction per W-tile index
        last_act_obs = None
        last_gp_obs = None
        last_eclaim = None
        wtts = {}  # live wtt tiles by tile index

        def load_tile(t):
            nonlocal last_act_obs, last_gp_obs
            ns, kt = divmod(t, KT)
            nlo = ns * NFREE
            gp_parity = t % 2 == 1  # odd W tiles load via the GpSimd queue
            # Observers anchor on the EXACT instruction whose tick the next
            # DMA's WAR needs: the transpose that read the recycled wsz slot.
            if t >= WSZ_BUFS:
                if gp_parity:
                    last_gp_obs = gp_observe(tr_hist[t - WSZ_BUFS], "gp clock")
                else:
                    last_act_obs = act_observe(tr_hist[t - WSZ_BUFS], "act clock")
            if ns == 0 and kt >= XS_BUFS:
                # x staging WAR: the xsb copy that read xs(kt-XS_BUFS).
                last_act_obs = act_observe(xcp_hist[kt - XS_BUFS], "act x clock")
            if ns == 0:
                # x prologue interleaved with strip 0 (Scalar queue).
                xs = xstg_pool.tile(
                    [P, MS], f32, name=f"xs{kt}", tag="xs", bufs=XS_BUFS
                )
                first = _swizzled_load(nc.scalar, xs, x_ap[:, kt * P : (kt + 1) * P])
                if last_act_obs is not None:
                    add_dep_helper(
                        first.ins, last_act_obs.ins, sync=False, reason="x after obs"
                    )
                _touch4(nc, xs)
                xsb = xstg_pool.tile([P, MS], bf16, name=f"xsb{kt}", tag="xsb", bufs=2)
                xcp = nc.vector.tensor_copy(out=xsb[:], in_=xs[:])  # ->bf16
                xcp_hist.append(xcp)
                nc.vector.transpose(xt[:, kt, :], xsb[:])
            # W tile: swizzle DMAs on one queue (alternating per tile), then
            # touch4 -> in-place bitwise sign -> strided-u16 transpose on DVE.
            wsz = wstg_pool.tile(
                [P, NFREE], f32, name=f"wsz_{t}", tag="wsz", bufs=WSZ_BUFS
            )
            first = _swizzled_load(
                nc.gpsimd if gp_parity else nc.scalar,
                wsz,
                w_ap[nlo : nlo + NFREE, kt * P : (kt + 1) * P],
            )
            pin = last_gp_obs if gp_parity else last_act_obs
            if pin is not None:
                add_dep_helper(first.ins, pin.ins, sync=False, reason="dma after obs")
            _touch4(nc, wsz)
            # in-place sign: (w & 0x80000000) | 0x3F800000 == +-1.0f. Reads
            # AND writes every staged byte, so the recycling DMA's deps
            # collapse into one DVE tick (<= the transpose read below).
            nc.vector.tensor_scalar(
                out=wsz[:].bitcast(u32),
                in0=wsz[:].bitcast(u32),
                scalar1=SIGN_AND32,
                scalar2=SIGN_OR32,
                op0=AND,
                op1=OR,
            )
            wtt = wstg_pool.tile(
                [P, NFREE], bf16, name=f"wtt_{t}", tag="wtt", bufs=WTT_BUFS
            )
            if t >= WTT_BUFS:
                # DVE observes PE past the matmuls that read the recycled wtt
                # slot, so the transpose keeps only its own-queue (sign) wait.
                dob = dve_observe(mm_last[t - WTT_BUFS], "dve sees pe")
            tr = nc.vector.transpose(
                wtt[:].bitcast(u16), wsz[:].bitcast(u16)[:, 1::2]
            )
            if t >= WTT_BUFS:
                add_dep_helper(
                    tr.ins, dob.ins, sync=False, reason="transpose after pe obs"
                )
            tr_hist.append(tr)
            wtts[t] = wtt

        def consume_tile(t):
            nonlocal last_eclaim
            ns, kt = divmod(t, KT)
            nlo = ns * NFREE
            if kt == 0:
                # bias enters PSUM first: rank-1 matmul, start=True clears
                # the bank; waits only bank mi's eviction copy (DVE).
                for mi in range(MT):
                    nc.tensor.matmul(
                        psums[mi][:],
                        ones_row[:],
                        bias_sgn[:, nlo : nlo + NFREE],
                        start=True,
                        stop=False,
                    )
            wtt = wtts.pop(t)
            last = kt == KT - 1
            for mi in range(MT):
                mm = nc.tensor.matmul(
                    psums[mi][:],
                    xt[:, kt, mi * P : (mi + 1) * P],
                    wtt[:],
                    start=False,
                    stop=last,
                )
            mm_last.append(mm)
            if last:
                # Staggered per-bank eviction into ot_big slices. Each bank's
                # out-DMA follows its OWN ACT observe (anchored on that
                # bank's copy), so no cross-copy scheduling assumption is
                # load-bearing: the DMA's data wait elides against a tick
                # that provably covers exactly the slice it reads.
                for mi in range(MT):
                    s = psums[mi][0:1, 0:1]
                    nc.vector.tensor_copy(out=s, in_=s)
                    cp = nc.vector.tensor_copy(
                        out=ot_big[:, mi, :], in_=psums[mi][:]
                    )
                    ecl = act_observe(cp, "eclaim")
                    di = nc.scalar.dma_start(
                        o_ap[mi * P : (mi + 1) * P, nlo : nlo + NFREE],
                        ot_big[:, mi, :],
                    )
                    add_dep_helper(
                        di.ins, ecl.ins, sync=False, reason="out after eclaim"
                    )

        for t in range(NT + SKEW):
            # consume first so a strip's eviction copies land in DVE program
            # order right after that strip's last transpose, not behind the
            # next strip's staging work.
            if t >= SKEW:
                consume_tile(t - SKEW)
            if t < NT:
                load_tile(t)


def build_module(m_shard=M_SHARD, k=K_FULL, n=N_FULL):
    nc = bass.Bass("TRN2", target_bir_lowering=False, debug=False)
    f32 = mybir.dt.float32
    x_d = nc.dram_tensor("x", [m_shard, k], f32, kind="ExternalInput")
    w_d = nc.dram_tensor("weight", [n, k], f32, kind="ExternalInput")
    b_d = nc.dram_tensor("bias", [n], f32, kind="ExternalInput")
    o_d = nc.dram_tensor("out", [m_shard, n], f32, kind="ExternalOutput")
    with SplitDrainTileContext(nc) as tc:
        bin_linear_tile_kernel(tc, x_d.ap(), w_d.ap(), b_d.ap(), o_d.ap())
    return nc


_NC_CACHE = {}


def _get_module():
    if "nc" not in _NC_CACHE:
        _NC_CACHE["nc"] = build_module()
    return _NC_CACHE["nc"]


def make_in_maps(x, weight, bias):
    x = np.ascontiguousarray(np.asarray(x, dtype=np.float32))
    weight = np.ascontiguousarray(np.asarray(weight, dtype=np.float32))
    bias = np.ascontiguousarray(np.asarray(bias, dtype=np.float32))
    return [
        {
            "x": x[i * M_SHARD : (i + 1) * M_SHARD],
            "weight": weight,
            "bias": bias,
        }
        for i in range(N_CORES)
    ]


def gather(results):
    return np.concatenate([results[i]["out"] for i in range(N_CORES)], axis=0)


def run(x, weight, bias, trace=False, **kw):
    """Run on the 8 NeuronCores; returns (out_full, BassKernelResults)."""
    nc = _get_module()
    in_maps = make_in_maps(x, weight, bias)
    res = run_bass_kernel_spmd(nc, in_maps, list(range(N_CORES)), trace=trace, **kw)
    return gather(res.results), res


def kernel(x, weight, bias):
    out, _ = run(x, weight, bias)
    return out


# revision 32
# speedup vs baseline: 1.0564x; 1.0474x over previous
"""BinLinear Trainium2 kernel: out = x @ sign(W)^T + sign(bias).

Full shapes: x [8192, 4096] f32, W [4096, 4096] f32, bias [4096] f32,
out [8192, 4096] f32. 8 NeuronCores, data-parallel on the token dim M:
core i gets x[1024*i:1024*(i+1)], full W/bias; host concatenates outputs.

Design (measured 0.996 ms vs the 1.54 ms hi/lo baseline; rel err 1.7e-3,
tolerance 2e-2):
  - ONE bf16 matmul pass: x rounded to bf16 (rel ~1e-3), sign(W) exact in
    bf16, fp32 PSUM accumulation. Per (kt, mi): moving = sign(W)^T tile
    [128,512], stationary = x^T tile [128,128]; per-strip rank-1 bias
    matmul (DVE-memset ones row x bit-tricked sign(bias) row) clears and
    seeds each PSUM bank.
  - Software-pipelined emission (skew 1, consume-before-load) so each
    strip's PSUM evictions land in DVE program order right after that
    strip's last transpose; per-bank staggered eviction keeps strip
    boundaries to ~1-2us of PE idle.
  - W pipeline per tile, all off the critical ACT path: 4 swizzled DMAs
    (3-dim APs; queue alternates Scalar/GpSimd per tile, never split within
    a tile - a recycled slot's WAW only collapses for same-queue writers),
    4 one-elem DVE lane touches, an in-place DVE bitwise sign
    ((w & 0x80000000) | 0x3F800000 == +-1.0f, reading every staged byte so
    the recycling DMA has a single collapsible dep), and a strided-u16
    32x32 stream transpose of the +-1.0f high half-words -> bf16 W^T.
  - x^T is built the same way during strip 0 (swizzle -> DVE round-copy ->
    transpose) into a resident [128, 32, 1024] bf16 tensor.
  - walrus allows ONE sync wait per instruction, and Tile emits waits for
    every tile-mediated dep it cannot prove covered by the engine's
    already-waited clock. All cross-engine coupling therefore runs through
    write-once observer scratches with forced sync edges (_observe):
    GpSimd/ACT observe the exact transpose whose staging slot their next
    DMA recycles; DVE observes the exact matmul whose wtt slot the next
    transpose overwrites; per-bank ACT eclaims observe each eviction copy
    so each out-DMA elides its data wait and keeps only its lane wait.
"""

import numpy as np

import concourse.bass as bass
import concourse.mybir as mybir
import concourse.tile as tile
from concourse.vector_clock import ScopedClock, VectorClock
from concourse.tile import add_dep_helper
from concourse.bass_utils import run_bass_kernel_spmd


class SplitDrainTileContext(tile.TileContext):
    """TileContext whose kernel-tail drain is split into several drain
    instructions. The stock tail emits ONE drain waiting on every active proc
    (engines + all DMA lanes, ~15 waits) which overflows the CTRL
    instruction's sync-wait slots in walrus codegen. Emitting the same waits
    across several drains (<= 4 waits each) is semantically identical: each
    drain's waits are satisfied in turn and the final state is 'everything
    quiesced'."""

    MAX_DRAIN_WAITS = 1

    def _drain_and_barrier(self, tick_clock, wait_clock):
        gc = tick_clock.global_clock
        n = len(gc)
        for lo in range(0, n, self.MAX_DRAIN_WAITS):
            vc = VectorClock()
            for p in range(lo, min(lo + self.MAX_DRAIN_WAITS, n)):
                if gc[p]:
                    vc.require_at_least(p, gc[p])
            drain_inst = self.nc.sync.drain()
            wait_clock.add_sem_waits(
                drain_inst.ins, ScopedClock({None: vc})
            )
        self.nc.all_engine_barrier()
        assert self.sems is not None
        popped = self.nc._tile_sem_poison_stack.pop()
        assert popped is self._sem_poison
        self.nc.clear_and_free_semaphores(list(self.sems.allocated().values()))
        self.nc.all_engine_barrier()


P = 128
NFREE = 512  # moving free dim per matmul (one PSUM bank of fp32)

M_FULL, K_FULL, N_FULL = 8192, 4096, 4096
N_CORES = 8
M_SHARD = M_FULL // N_CORES

# sign bit-trick masks
SIGN_AND = 0x8000
SIGN_OR = 0x3F80  # 1.0 in bf16
SIGN_AND32 = 0x80000000
SIGN_OR32 = 0x3F800000  # 1.0 in f32


def _swizzled_load(engine, sbuf_tile, dram_ap):
    """Load dram_ap ([R, 128] slice) into sbuf_tile [128, R] block-swizzled so
    that a DVE 32x32 stream transpose of sbuf_tile yields dram_ap.T.

    Pre-DVE we need:  sbuf[32g+a, 32b+c] = dram[32b+a, 32g+c]
    so post-DVE:      out[32g+a, 32b+c] = dram[32b+c, 32g+a] = dram.T[p, f].

    DMA access patterns are limited to 3 dims, so issue one DMA per
    partition-group g (source dims [a, b, c], 128-byte contiguous runs).

    ALL four DMAs must come from the SAME queue: a recycled slot's new DMA
    carries WAW waits vs the old tile's writers, and only same-queue lane
    ticks are covered by the issuing queue's own lane-wait chain (cross-queue
    lane sems would each cost a sync-wait slot the DMA doesn't have).
    """
    first = None
    for g in range(4):
        di = engine.dma_start(
            sbuf_tile[32 * g : 32 * (g + 1), :],
            dram_ap[:, 32 * g : 32 * (g + 1)].rearrange("(b a) c -> a b c", a=32),
        )
        if first is None:
            first = di
    return first


def _touch4(nc, sbuf_tile):
    """In-place 1-element DVE copies, one per partition group. Each waits on
    one of the 4 swizzle DMAs, advancing the DVE's observed semaphore ticks so
    the full-width consumer that follows needs no waits of its own (the HW
    allows only a few sync-wait commands per instruction)."""
    for g in range(4):
        s = sbuf_tile[32 * g : 32 * (g + 1), 0:1]
        nc.vector.tensor_copy(out=s, in_=s)


def _observe(eng_memset_or_act, scr, anchor_inst, reason):
    """Advance a queue's observed clock past `anchor_inst` without touching
    any real data tile: a write-once 1-elem scratch write plus a forced
    sync edge. The write-once target means no WAW; the single forced wait is
    the instruction's only one, and later same-queue instructions elide any
    dep at or before the anchor's tick. Returns the observer instruction."""
    inst = eng_memset_or_act(scr)
    add_dep_helper(inst.ins, anchor_inst.ins, sync=True, reason=reason)
    return inst


def bin_linear_tile_kernel(tc, x_ap, w_ap, b_ap, o_ap):
    nc = tc.nc
    f32 = mybir.dt.float32
    bf16 = mybir.dt.bfloat16
    u16 = mybir.dt.uint16
    u32 = mybir.dt.uint32
    AND = mybir.AluOpType.bitwise_and
    OR = mybir.AluOpType.bitwise_or
    COPY = mybir.ActivationFunctionType.Copy

    MS, K = x_ap.shape  # m per core, contraction
    N = w_ap.shape[0]
    KT = K // P  # k tiles
    MT = MS // P  # m tiles (psum banks used per n-strip)
    NS = N // NFREE  # n strips
    NT = NS * KT  # total W tiles
    SKEW = 1  # load-ahead: W tile t is loaded SKEW iterations before its MMs
    # (small on purpose: the DMA queues already run ahead via the staging
    # bufs; a bigger skew just pushes each strip's eviction copies later in
    # DVE program order, stretching the strip-boundary PE gap and
    # re-throttling HAM.)
    WSZ_BUFS = 10  # even: a recycled slot's old DMA writers are same-queue
    WTT_BUFS = 24
    XS_BUFS = 4  # even: x tiles alternate Scalar/GpSimd by kt parity
    assert MT <= 8, "psum accumulators exceed the 8 PSUM banks"

    with (
        tc.tile_pool(name="xt", bufs=1) as xt_pool,
        tc.tile_pool(name="xstg", bufs=2) as xstg_pool,
        tc.tile_pool(name="wstg", bufs=2) as wstg_pool,
        tc.tile_pool(name="outp", bufs=1) as out_pool,
        tc.tile_pool(name="bias", bufs=1) as bias_pool,
        tc.tile_pool(name="obs", bufs=1) as obs_pool,
        tc.tile_pool(name="psum", bufs=8, space="PSUM") as psum_pool,
    ):
        # Write-once observer scratches (see _observe). Unique cells: a
        # rotating scratch's WAW would cost a second wait on engines whose
        # own-sem clock never advances (Pool/ACT).
        nobs = [0]

        def gp_observe(anchor, reason):
            scr = obs_pool.tile([1, 1], f32, name=f"gsc{nobs[0]}")
            nobs[0] += 1
            return _observe(
                lambda s: nc.gpsimd.memset(s[:], 0.0), scr, anchor, reason
            )

        def dve_observe(anchor, reason):
            scr = obs_pool.tile([1, 1], f32, name=f"dsc{nobs[0]}")
            nobs[0] += 1
            return _observe(
                lambda s: nc.vector.memset(s[:], 0.0), scr, anchor, reason
            )

        # --- bias: sign via the DVE bit trick; rank-1 matmul operands.
        bstg = bias_pool.tile([1, N], f32, name="bstg")
        nc.sync.dma_start(bstg[:], b_ap[None, :])
        s = bstg[0:1, 0:1]
        nc.vector.tensor_copy(out=s, in_=s)
        bias_sgn = bias_pool.tile([1, N], bf16, name="bias_sgn")
        nc.vector.tensor_scalar(
            out=bias_sgn[:].bitcast(u16),
            in0=bstg[:].bitcast(u16)[:, 1::2],
            scalar1=SIGN_AND,
            scalar2=SIGN_OR,
            op0=AND,
            op1=OR,
        )
        ones_row = bias_pool.tile([1, P], bf16, name="ones_row")
        nc.vector.memset(ones_row[:], 1.0)
        zero_row = bias_pool.tile([1, NFREE], bf16, name="zero_row")
        nc.vector.memset(zero_row[:], 0.0)

        def act_observe(anchor, reason):
            # ACT observer: 1-elem activation copy from the never-rewritten
            # ones_row into a write-once scratch; the forced DVE edge merges
            # with the (ancient) ones_row RAW into a single DVE wait.
            scr = obs_pool.tile([1, 1], f32, name=f"asc{nobs[0]}")
            nobs[0] += 1
            inst = nc.scalar.activation(scr[:], ones_row[0:1, 0:1], COPY)
            add_dep_helper(inst.ins, anchor.ins, sync=True, reason=reason)
            return inst

        # x^T resident: [128, KT, MS] bf16
        xt = xt_pool.tile([P, KT, MS], bf16, name="xt")
        # out staging: one [128, MT*NFREE] tile per strip, written by the MT
        # eviction copies, drained by ONE 3D out-DMA (dst dims [mi, p, n]).
        ot_big = out_pool.tile([P, MT, NFREE], f32, name="ot_big")

        psums = [
            psum_pool.tile([P, NFREE], f32, name=f"psum_{mi}", tag="acc")
            for mi in range(MT)
        ]

        tr_hist = []  # wtt transpose instruction per W-tile index
        xcp_hist = []  # xsb-copy instruction per x tile
        mm_last = []  # last matmul instruction per W-tile index
        last_act_obs = None
        last_gp_obs = None
        last_eclaim = None
        wtts = {}  # live wtt tiles by tile index

        def load_tile(t):
            nonlocal last_act_obs, last_gp_obs
            ns, kt = divmod(t, KT)
            nlo = ns * NFREE
            gp_parity = t % 2 == 1  # odd W tiles load via the GpSimd queue
            # Observers anchor on the EXACT instruction whose tick the next
            # DMA's WAR needs: the transpose that read the recycled wsz slot.
            if t >= WSZ_BUFS:
                if gp_parity:
                    last_gp_obs = gp_observe(tr_hist[t - WSZ_BUFS], "gp clock")
                else:
                    last_act_obs = act_observe(tr_hist[t - WSZ_BUFS], "act clock")
            if ns == 0 and kt >= XS_BUFS:
                # x staging WAR: the xsb copy that read xs(kt-XS_BUFS); the
                # observer lives on whichever queue loads this x tile.
                if kt % 2 == 1:
                    last_gp_obs = gp_observe(xcp_hist[kt - XS_BUFS], "gp x clock")
                else:
                    last_act_obs = act_observe(xcp_hist[kt - XS_BUFS], "act x clock")
            if ns == 0:
                # x prologue interleaved with strip 0; tiles alternate
                # queues by kt parity so neither descriptor queue paces it.
                xs = xstg_pool.tile(
                    [P, MS], f32, name=f"xs{kt}", tag="xs", bufs=XS_BUFS
                )
                x_gp = kt % 2 == 1
                first = _swizzled_load(
                    nc.gpsimd if x_gp else nc.scalar,
                    xs,
                    x_ap[:, kt * P : (kt + 1) * P],
                )
                xpin = last_gp_obs if x_gp else last_act_obs
                if xpin is not None:
                    add_dep_helper(
                        first.ins, xpin.ins, sync=False, reason="x after obs"
                    )
                _touch4(nc, xs)
                xsb = xstg_pool.tile([P, MS], bf16, name=f"xsb{kt}", tag="xsb", bufs=2)
                xcp = nc.vector.tensor_copy(out=xsb[:], in_=xs[:])  # ->bf16
                xcp_hist.append(xcp)
                nc.vector.transpose(xt[:, kt, :], xsb[:])
            # W tile: swizzle DMAs on one queue (alternating per tile), then
            # touch4 -> in-place bitwise sign -> strided-u16 transpose on DVE.
            wsz = wstg_pool.tile(
                [P, NFREE], f32, name=f"wsz_{t}", tag="wsz", bufs=WSZ_BUFS
            )
            first = _swizzled_load(
                nc.gpsimd if gp_parity else nc.scalar,
                wsz,
                w_ap[nlo : nlo + NFREE, kt * P : (kt + 1) * P],
            )
            pin = last_gp_obs if gp_parity else last_act_obs
            if pin is not None:
                add_dep_helper(first.ins, pin.ins, sync=False, reason="dma after obs")
            _touch4(nc, wsz)
            # in-place sign: (w & 0x80000000) | 0x3F800000 == +-1.0f. Reads
            # AND writes every staged byte, so the recycling DMA's deps
            # collapse into one DVE tick (<= the transpose read below).
            nc.vector.tensor_scalar(
                out=wsz[:].bitcast(u32),
                in0=wsz[:].bitcast(u32),
                scalar1=SIGN_AND32,
                scalar2=SIGN_OR32,
                op0=AND,
                op1=OR,
            )
            wtt = wstg_pool.tile(
                [P, NFREE], bf16, name=f"wtt_{t}", tag="wtt", bufs=WTT_BUFS
            )
            if t >= WTT_BUFS:
                # DVE observes PE past the matmuls that read the recycled wtt
                # slot, so the transpose keeps only its own-queue (sign) wait.
                dob = dve_observe(mm_last[t - WTT_BUFS], "dve sees pe")
            tr = nc.vector.transpose(
                wtt[:].bitcast(u16), wsz[:].bitcast(u16)[:, 1::2]
            )
            if t >= WTT_BUFS:
                add_dep_helper(
                    tr.ins, dob.ins, sync=False, reason="transpose after pe obs"
                )
            tr_hist.append(tr)
            wtts[t] = wtt

        def consume_tile(t):
            nonlocal last_eclaim
            ns, kt = divmod(t, KT)
            nlo = ns * NFREE
            if kt == 0:
                # bias enters PSUM first: rank-1 matmul, start=True clears
                # the bank; waits only bank mi's eviction copy (DVE).
                for mi in range(MT):
                    nc.tensor.matmul(
                        psums[mi][:],
                        ones_row[:],
                        bias_sgn[:, nlo : nlo + NFREE],
                        start=True,
                        stop=False,
                    )
            wtt = wtts.pop(t)
            last = kt == KT - 1
            for mi in range(MT):
                mm = nc.tensor.matmul(
                    psums[mi][:],
                    xt[:, kt, mi * P : (mi + 1) * P],
                    wtt[:],
                    start=False,
                    stop=last,
                )
            mm_last.append(mm)
            if 1 <= kt <= KT - 2:
                # Warm-keepers: rank-1 matmuls accumulating exact +0.0 into an
                # active bank. Zero dependencies, so they run back-to-back in
                # the PE FIFO whenever the next tile's transpose isn't ready,
                # keeping the HAM activity window busy (cold-throttle was
                # ~40% of PE time). Cost if never needed: ~213ns each.
                for j in range(4 if ns == 0 else 2):
                    nc.tensor.matmul(
                        psums[(kt + j) % MT][:],
                        ones_row[:],
                        zero_row[:],
                        start=False,
                        stop=False,
                    )
            if last:
                # Staggered per-bank eviction into ot_big slices. Each bank's
                # out-DMA follows its OWN ACT observe (anchored on that
                # bank's copy), so no cross-copy scheduling assumption is
                # load-bearing: the DMA's data wait elides against a tick
                # that provably covers exactly the slice it reads.
                for mi in range(MT):
                    s = psums[mi][0:1, 0:1]
                    nc.vector.tensor_copy(out=s, in_=s)
                    cp = nc.vector.tensor_copy(
                        out=ot_big[:, mi, :], in_=psums[mi][:]
                    )
                    ecl = act_observe(cp, "eclaim")
                    di = nc.scalar.dma_start(
                        o_ap[mi * P : (mi + 1) * P, nlo : nlo + NFREE],
                        ot_big[:, mi, :],
                    )
                    add_dep_helper(
                        di.ins, ecl.ins, sync=False, reason="out after eclaim"
                    )

        for t in range(NT + SKEW):
            # consume first so a strip's eviction copies land in DVE program
            # order right after that strip's last transpose, not behind the
            # next strip's staging work.
            if t >= SKEW:
                consume_tile(t - SKEW)
            if t < NT:
                load_tile(t)


def build_module(m_shard=M_SHARD, k=K_FULL, n=N_FULL):
    nc = bass.Bass("TRN2", target_bir_lowering=False, debug=False)
    f32 = mybir.dt.float32
    x_d = nc.dram_tensor("x", [m_shard, k], f32, kind="ExternalInput")
    w_d = nc.dram_tensor("weight", [n, k], f32, kind="ExternalInput")
    b_d = nc.dram_tensor("bias", [n], f32, kind="ExternalInput")
    o_d = nc.dram_tensor("out", [m_shard, n], f32, kind="ExternalOutput")
    with SplitDrainTileContext(nc) as tc:
        bin_linear_tile_kernel(tc, x_d.ap(), w_d.ap(), b_d.ap(), o_d.ap())
    return nc


_NC_CACHE = {}


def _get_module():
    if "nc" not in _NC_CACHE:
        _NC_CACHE["nc"] = build_module()
    return _NC_CACHE["nc"]


def make_in_maps(x, weight, bias):
    x = np.ascontiguousarray(np.asarray(x, dtype=np.float32))
    weight = np.ascontiguousarray(np.asarray(weight, dtype=np.float32))
    bias = np.ascontiguousarray(np.asarray(bias, dtype=np.float32))
    return [
        {
            "x": x[i * M_SHARD : (i + 1) * M_SHARD],
            "weight": weight,
            "bias": bias,
        }
        for i in range(N_CORES)
    ]


def gather(results):
    return np.concatenate([results[i]["out"] for i in range(N_CORES)], axis=0)


def run(x, weight, bias, trace=False, **kw):
    """Run on the 8 NeuronCores; returns (out_full, BassKernelResults)."""
    nc = _get_module()
    in_maps = make_in_maps(x, weight, bias)
    res = run_bass_kernel_spmd(nc, in_maps, list(range(N_CORES)), trace=trace, **kw)
    return gather(res.results), res


def kernel(x, weight, bias):
    out, _ = run(x, weight, bias)
    return out
